# revision 24
# baseline (speedup 1.0000x reference)
"""Chamfer distance L2 kernel for Trainium2, 8 NeuronCores.

Problem: xyz1, xyz2 [B=4, N=8192, 3] fp32. Output: scalar
mean_i(min_j ||x1_i - x2_j||^2) + mean_j(min_i ||x1_i - x2_j||^2).

Decomposition: 8 independent jobs = (batch, direction), one per NeuronCore.
Each job: for 8192 query points, exact min squared distance to 8192
candidates.

Algorithm (exact, 2-round candidate pruning):
  * Host orders each job's queries with a k-d median partition (leaves of
    LEAF=8) so each "unit" of BQ=32 consecutive queries is 4 compact
    sub-boxes.
  * For each unit, host gathers the W=288 candidates nearest to the unit
    (by min squared distance to its leaf bboxes -- a lower bound on any
    query-candidate distance) and records, per leaf, the smallest bound
    among NON-gathered candidates (the leaf's coverage radius rcov).
  * Device (round 1) computes per-query min over the gathered candidates.
    Four units share one matmul slot: four K=15 column-tiled matmuls
    (tile_position=(0,32h), concurrent on the PE array) emit pairwise
    squared distances for 4x32 queries into one PSUM bank (bf16 hi/lo
    compensated products accumulated in fp32; the query-side |a|^2 term
    is constant per row and added on the host after the min, which also
    lets max(.,0) commute out). VectorE reduce_min over a [128, GRP, W]
    view produces the row mins, 4 slots per fused reduce.
  * Host verifies per query: if device_min + |a|^2 + pad(q) <= rcov(leaf),
    every non-gathered candidate is provably farther than the best found
    -> exact. pad(q) soundly bounds the device arithmetic error
    (~2.5e-5*|a|^2 + 2e-5). Queries failing the test ("stragglers") are
    regrouped; all candidates within their upper-bound balls (bounded via
    sub-bboxes again) are chunked into W-sized units and run through a
    second, smaller compiled NEFF; host min-combines. Round 2 is
    conclusive -- every candidate that could beat the round-1 bound is
    included -- so no further verification is needed.

The device does all distance arithmetic; the host only sorts/gathers by
coordinate bounds and combines results.

Pairwise matmul row content (K=15):
   k 0..2 : (-2*a_hi) * b_hi      k 3..5 : (-2*a_hi) * b_lo
   k 6..8 : (-2*a_lo) * b_hi      k 9..11: (-2*a_lo) * b_lo
   k12..14: 1 * sqB_{hi,lo,lo2}
bf16*bf16 products are exact in fp32, so the dominant error is the dropped
sub-bf16 residue of the splits, ~1e-4 absolute on d^2.
"""

import numpy as np
import ml_dtypes

import concourse.bass as bass
import concourse.tile as tile
from concourse import bacc, mybir
from concourse.bass_utils import run_bass_kernel_spmd

BF16 = ml_dtypes.bfloat16
F32 = np.float32

K = 15            # augmented contraction rows
W = 288           # candidates per 32-query block ("unit")
BQ = 32           # queries per unit; four units share one matmul slot via
                  # PE column-tiling (tile_position=(0, 32*h))
UPB = 128 // BQ   # units per slot
PSW = 512         # PSUM bank stride in fp32 elements (one matmul <= 1 bank)
NSLOT1 = 64       # slots per core, round-1 NEFF (= 256 units)
NSLOT2 = 12       # slots per core, straggler NEFF (= 48 units)
GRP = 4           # slots fused per DMA + reduce (4 PSUM banks)
LEAF = 8          # k-d leaf size -> 4 sub-bboxes per 32-query unit
N_CORES = 8

# Sound per-query bound on device pairwise-d^2 arithmetic error:
# split residues ~2^-16*|a||b| + fp32 PSUM accumulation ~K*2^-23*|partials|.
PAD_SCALE = 2.5e-5
PAD_ABS = 2e-5


def _pad_q(sqA):
    return PAD_SCALE * sqA + PAD_ABS


# --------------------------------------------------------------------------
# Device program (static NEFFs, SPMD on 8 cores)
# --------------------------------------------------------------------------

def build_kernel(nslot):
    nc = bacc.Bacc("TRN2", target_bir_lowering=False, debug=False)

    lhsT_d = nc.dram_tensor("lhsT", [K, nslot * 128], mybir.dt.bfloat16,
                            kind="ExternalInput")
    rhs_d = nc.dram_tensor("rhs", [nslot // GRP, K, GRP * UPB * W],
                           mybir.dt.bfloat16, kind="ExternalInput")
    out_d = nc.dram_tensor("mins", [128, nslot], mybir.dt.float32,
                           kind="ExternalOutput")

    with tile.TileContext(nc) as tc:
        with (
            tc.tile_pool(name="io", bufs=1) as io_pool,
            tc.tile_pool(name="rh", bufs=4) as rh_pool,
            tc.tile_pool(name="ps", bufs=2, space=bass.MemorySpace.PSUM) as ps_pool,
        ):
            lhsT_s = io_pool.tile([K, nslot * 128], mybir.dt.bfloat16)
            nc.sync.dma_start(lhsT_s[:], lhsT_d[:])
            mins_all = io_pool.tile([128, nslot], mybir.dt.float32)

            for g in range(nslot // GRP):
                rt = rh_pool.tile([K, GRP * UPB * W], mybir.dt.bfloat16)
                nc.sync.dma_start(rt[:], rhs_d[g])
                # GRP banks; slot s in bank s, cols 0..W of the bank; the
                # two 64-query units of a slot land on partition halves via
                # PE column-tiling with their own rhs windows.
                ps = ps_pool.tile([128, GRP * PSW], mybir.dt.float32)
                for s in range(GRP):
                    m = g * GRP + s
                    for h in range(UPB):
                        nc.tensor.matmul(
                            ps[h * BQ : (h + 1) * BQ, s * PSW : s * PSW + W],
                            lhsT_s[:, m * 128 + h * BQ : m * 128 + (h + 1) * BQ],
                            rt[:, (s * UPB + h) * W : (s * UPB + h + 1) * W],
                            tile_position=(0, h * BQ),
                        )
                nc.vector.tensor_reduce(
                    mins_all[:, g * GRP : (g + 1) * GRP],
                    ps[:].rearrange("p (s n) -> p s n", n=PSW)[:, :, 0:W],
                    axis=mybir.AxisListType.X,
                    op=mybir.AluOpType.min,
                )

            nc.sync.dma_start(out_d[:], mins_all[:])

    nc.compile()
    return nc


_NC_CACHE = {}


def _get_nc(nslot):
    if nslot not in _NC_CACHE:
        _NC_CACHE[nslot] = build_kernel(nslot)
    return _NC_CACHE[nslot]


class _PjrtRunner:
    """Compile-once PJRT executor for one NEFF across the 8 cores.

    Mirrors bass2jax.run_bass_via_pjrt's multi-core path but holds the
    jitted shard_map so repeated waves skip XLA re-compilation.
    """

    def __init__(self, nc):
        import jax
        from concourse import bass2jax

        bass2jax.install_neuronx_cc_hook()
        self._jax = jax
        partition_name = (nc.partition_id_tensor.name
                          if nc.partition_id_tensor else None)
        in_names = []
        out_names = []
        out_avals = []
        zero_outs = []
        for alloc in nc.m.functions[0].allocations:
            if not isinstance(alloc, mybir.MemoryLocationSet):
                continue
            name = alloc.memorylocations[0].name
            if alloc.kind == "ExternalInput":
                if name != partition_name:
                    in_names.append(name)
            elif alloc.kind == "ExternalOutput":
                out_names.append(name)
                shape = tuple(alloc.tensor_shape)
                dtype = mybir.dt.np(alloc.dtype)
                out_avals.append(jax.core.ShapedArray(shape, dtype))
                zero_outs.append(np.zeros(shape, dtype))
        self.in_names = in_names
        self.out_names = out_names
        self.out_avals = out_avals
        self.zero_outs = zero_outs
        n_params = len(in_names)
        n_outs = len(out_names)
        all_in_names = list(in_names) + list(out_names)
        if partition_name is not None:
            all_in_names.append(partition_name)
        all_in_names = tuple(all_in_names)

        def _body(*args):
            operands = list(args)
            if partition_name is not None:
                operands.append(bass2jax.partition_id_tensor())
            outs = bass2jax._bass_exec_p.bind(
                *operands,
                out_avals=tuple(out_avals),
                in_names=all_in_names,
                out_names=tuple(out_names),
                lowering_input_output_aliases=(),
                sim_require_finite=True,
                sim_require_nnan=True,
                nc=nc,
            )
            return tuple(outs)

        devices = jax.devices()[:N_CORES]
        mesh = bass2jax.Mesh(np.asarray(devices), ("core",))
        P = bass2jax.PartitionSpec
        self._fn = jax.jit(
            bass2jax.shard_map(
                _body,
                mesh=mesh,
                in_specs=(P("core"),) * (n_params + n_outs),
                out_specs=(P("core"),) * n_outs,
                check_rep=False,
            ),
            donate_argnums=tuple(range(n_params, n_params + n_outs)),
            keep_unused=True,
        )

    def __call__(self, in_maps):
        np_ = np
        concat_in = [
            np_.concatenate([np_.asarray(m[name]) for m in in_maps], axis=0)
            for name in self.in_names
        ]
        concat_zeros = [
            np_.zeros((N_CORES * z.shape[0], *z.shape[1:]), z.dtype)
            for z in self.zero_outs
        ]
        out_arrs = self._fn(*concat_in, *concat_zeros)
        return [
            {
                name: np_.asarray(out_arrs[i]).reshape(
                    N_CORES, *self.out_avals[i].shape)[c]
                for i, name in enumerate(self.out_names)
            }
            for c in range(N_CORES)
        ]


_RUNNER_CACHE = {}


def _get_runner(nslot):
    if nslot not in _RUNNER_CACHE:
        _RUNNER_CACHE[nslot] = _PjrtRunner(_get_nc(nslot))
    return _RUNNER_CACHE[nslot]


class _WaveResults:
    def __init__(self, results):
        self.results = results


def run_wave(in_maps, nslot=NSLOT1, trace=False, **kw):
    if trace or kw:
        nc = _get_nc(nslot)
        return run_bass_kernel_spmd(nc, in_maps, list(range(N_CORES)),
                                    trace=trace, **kw)
    return _WaveResults(_get_runner(nslot)(in_maps))


# --------------------------------------------------------------------------
# Host-side prep
# --------------------------------------------------------------------------

def _split2(x):
    h = x.astype(BF16)
    l = (x - h.astype(F32)).astype(BF16)
    return h, l


def kd_order(P, leaf=LEAF):
    """Permutation grouping points into contiguous compact leaves of `leaf`."""
    out = []

    def rec(ids):
        if len(ids) <= leaf:
            out.append(ids)
            return
        pts = P[ids]
        ax = int(np.argmax(pts.max(0) - pts.min(0)))
        k = len(ids) // 2
        part = np.argpartition(pts[:, ax], k)
        rec(ids[part[:k]])
        rec(ids[part[k:]])

    rec(np.arange(len(P)))
    return np.concatenate(out)


_LEAF_D2_JIT = {}


def _leaf_d2_impl(lo, hi, B):
    import jax.numpy as jnp

    c = jnp.clip(B.T[:, None, :], lo.T[:, :, None], hi.T[:, :, None])
    t = B.T[:, None, :] - c                   # [3, nleaf, ncand]
    return (t * t).sum(0) * np.float32(1.0 - 1e-5)


def leaf_d2(q32, B32, leaf=LEAF):
    """[nleaf, ncand] fp32 lower bounds on min squared query-candidate dist.

    q32 is padded (by repeating the last point) to a multiple of `leaf`;
    the result is scaled by (1-1e-5) so fp32 rounding can never make it
    exceed the true distance.
    """
    import jax

    n = len(q32)
    if n % leaf:
        pad = leaf - n % leaf
        q32 = np.concatenate([q32, np.repeat(q32[-1:], pad, 0)])
    L = q32.reshape(-1, leaf, 3)
    lo = L.min(1)
    hi = L.max(1)
    key = (len(lo), len(B32))
    if key not in _LEAF_D2_JIT:
        cpu = jax.devices("cpu")[0]
        _LEAF_D2_JIT[key] = jax.jit(_leaf_d2_impl, device=cpu)
    return np.asarray(_LEAF_D2_JIT[key](lo, hi, B32))


class Job:
    """Host state for one (queries, candidates) job."""

    def __init__(self, Aq, Bc):
        self.N = len(Aq)
        self.order = kd_order(Aq)
        A = Aq[self.order]
        self.A32 = A
        self.B32 = Bc
        self.Ad = A.astype(np.float64)

        ah, al = _split2(A)
        m2ah = (ah.astype(F32) * -2.0).astype(BF16)
        m2al = (al.astype(F32) * -2.0).astype(BF16)
        L = np.empty((K, self.N), BF16)
        L[0:3] = m2ah.T
        L[3:6] = m2ah.T
        L[6:9] = m2al.T
        L[9:12] = m2al.T
        L[12:15] = np.ones((3, self.N), BF16)
        self.Lrows = L

        bh, bl = _split2(Bc)
        sqB = (Bc.astype(np.float64) ** 2).sum(-1).astype(F32)
        s0 = sqB.astype(BF16)
        r = sqB - s0.astype(F32)
        s1 = r.astype(BF16)
        s2 = (r - s1.astype(F32)).astype(BF16)
        R = np.empty((K, len(Bc)), BF16)
        R[0:3] = bh.T
        R[3:6] = bl.T
        R[6:9] = bh.T
        R[9:12] = bl.T
        R[12] = s0
        R[13] = s1
        R[14] = s2
        self.Rrows = R

        self.sqA = (self.Ad ** 2).sum(-1)  # permuted order, float64
        self.mins = np.full(self.N, np.inf)  # device value: d2 - sqA

        # Round-1 gather: per 64-query unit, W nearest-by-leaf-bbox
        # candidates; per leaf, coverage radius = min bound among
        # non-gathered.
        nblk = self.N // BQ
        nsub = BQ // LEAF
        d2 = leaf_d2(self.A32, self.B32)        # [nblk*nsub, ncand]
        ncand = len(self.B32)
        d2r = d2.reshape(nblk, nsub, ncand)
        d2b = d2r.min(1)                        # [nblk, ncand]
        part = np.argpartition(d2b, W, axis=1)
        self.sel = part[:, :W].copy()
        mask = np.zeros((nblk, ncand), bool)
        np.put_along_axis(mask, self.sel, True, axis=1)
        masked = np.where(mask[:, None, :], np.float32(np.inf), d2r)
        self.rcov = masked.min(2).reshape(-1).astype(np.float64)

    def round1_units(self):
        return [
            (np.arange(m * BQ, (m + 1) * BQ), self.sel[m])
            for m in range(self.N // BQ)
        ]

    def absorb(self, qidx, vals):
        np.minimum.at(self.mins, qidx, vals.astype(np.float64))

    def stragglers(self):
        """Per-query coverage check after round 1."""
        ub2 = np.maximum(self.mins + self.sqA, 0.0) + _pad_q(self.sqA)
        return np.where(ub2 > np.repeat(self.rcov, LEAF))[0]

    def round2_units(self, strag):
        """Conclusive follow-up units for straggler queries."""
        units = []
        if len(strag) == 0:
            return units
        sord = strag[kd_order(self.A32[strag])]
        for m0 in range(0, len(sord), BQ):
            ids = sord[m0 : m0 + BQ]
            d2b = leaf_d2(self.A32[ids], self.B32).min(0)
            ub2max = (np.maximum(self.mins[ids] + self.sqA[ids], 0.0)
                      + _pad_q(self.sqA[ids])).max()
            need = np.where(d2b <= ub2max)[0]
            if len(need) == 0:
                continue
            for c0 in range(0, len(need), W):
                cand = need[c0 : c0 + W]
                if len(cand) < W:
                    cand = np.concatenate(
                        [cand, np.full(W - len(cand), cand[0], np.int64)])
                units.append((ids, cand))
        return units


def _assemble_core(units, nslot):
    """Build one core's in_map from up to `2*nslot` (job, qidx, cand) units.

    Unit u maps to slot u//UPB, partition quarter u%UPB.
    """
    lhsT = np.zeros((K, nslot * 128), BF16)
    rhs = np.zeros((nslot // GRP, K, GRP * UPB * W), BF16)
    meta = []
    for u, (job, qidx, cand) in enumerate(units):
        s, h = divmod(u, UPB)
        ncol = len(qidx)
        c0 = s * 128 + h * BQ
        lhsT[:, c0 : c0 + ncol] = job.Lrows[:, qidx]
        g, r = divmod(s, GRP)
        rhs[g, :, (r * UPB + h) * W : (r * UPB + h + 1) * W] = job.Rrows[:, cand]
        meta.append((job, qidx, s, h))
    return {"lhsT": lhsT, "rhs": rhs}, meta


def _run_waves(all_units, nslot, trace=False):
    """Pack units onto cores, run as many 8-core waves as needed."""
    per_core = UPB * nslot
    per_wave = N_CORES * per_core
    for w0 in range(0, len(all_units), per_wave):
        wave = all_units[w0 : w0 + per_wave]
        in_maps = []
        metas = []
        for c in range(N_CORES):
            cunits = wave[c * per_core : (c + 1) * per_core]
            im, meta = _assemble_core(cunits, nslot)
            in_maps.append(im)
            metas.append(meta)
        res = run_wave(in_maps, nslot=nslot, trace=trace)
        for c in range(N_CORES):
            mins = res.results[c]["mins"]  # [128, nslot]
            for job, qidx, s, h in metas[c]:
                job.absorb(qidx, mins[h * BQ : h * BQ + len(qidx), s])


def kernel(xyz1, xyz2):
    xyz1 = np.asarray(xyz1, F32)
    xyz2 = np.asarray(xyz2, F32)
    nb = xyz1.shape[0]

    jobs = []
    for b in range(nb):
        jobs.append(Job(xyz1[b], xyz2[b]))
        jobs.append(Job(xyz2[b], xyz1[b]))

    # Round 1: job j's 128 units on core j (unit list is job-major)
    units1 = [(j, q, c) for j in jobs for q, c in j.round1_units()]
    _run_waves(units1, NSLOT1)

    # Round 2: conclusive straggler units (typically one short wave)
    units2 = [(j, q, c) for j in jobs for q, c in j.round2_units(j.stragglers())]
    if units2:
        nslot = NSLOT2 if len(units2) <= N_CORES * UPB * NSLOT2 else NSLOT1
        _run_waves(units2, nslot)

    total = 0.0
    for j in jobs:
        d = np.maximum(j.mins + j.sqA, 0.0)
        total += d.mean() / nb
    return np.asarray(total, dtype=F32)


# revision 25
# speedup vs baseline: 1.1511x; 1.1511x over previous
"""Chamfer distance L2 kernel for Trainium2, 8 NeuronCores.

Problem: xyz1, xyz2 [B=4, N=8192, 3] fp32. Output: scalar
mean_i(min_j ||x1_i - x2_j||^2) + mean_j(min_i ||x1_i - x2_j||^2).

Decomposition: 8 independent jobs = (batch, direction), one per NeuronCore.
Each job: for 8192 query points, exact min squared distance to 8192
candidates.

Algorithm (exact, 2-round candidate pruning):
  * Host orders each job's queries with a k-d median partition (leaves of
    LEAF=8) so each "unit" of BQ=32 consecutive queries is 4 compact
    sub-boxes.
  * For each unit, host gathers the W=192 candidates nearest to the unit
    (by min squared distance to its leaf bboxes -- a lower bound on any
    query-candidate distance) and records, per leaf, the smallest bound
    among NON-gathered candidates (the leaf's coverage radius rcov).
  * Device (round 1) computes per-query min over the gathered candidates.
    Four units share one matmul slot: four K=15 column-tiled matmuls
    (tile_position=(0,32h), concurrent on the PE array) emit pairwise
    squared distances for 4x32 queries into one PSUM bank (bf16 hi/lo
    compensated products accumulated in fp32; the query-side |a|^2 term
    is constant per row and added on the host after the min, which also
    lets max(.,0) commute out). VectorE reduce_min over a [128, GRP, W]
    view produces the row mins, 4 slots per fused reduce.
  * Host verifies per query: if device_min + |a|^2 + pad(q) <= rcov(leaf),
    every non-gathered candidate is provably farther than the best found
    -> exact. pad(q) soundly bounds the device arithmetic error
    (~2.5e-5*|a|^2 + 2e-5). Queries failing the test ("stragglers") are
    regrouped; all candidates within their upper-bound balls (bounded via
    sub-bboxes again) are chunked into W-sized units and run through a
    second, smaller compiled NEFF; host min-combines. Round 2 is
    conclusive -- every candidate that could beat the round-1 bound is
    included -- so no further verification is needed.

The device does all distance arithmetic; the host only sorts/gathers by
coordinate bounds and combines results.

Pairwise matmul row content (K=15):
   k 0..2 : (-2*a_hi) * b_hi      k 3..5 : (-2*a_hi) * b_lo
   k 6..8 : (-2*a_lo) * b_hi      k 9..11: (-2*a_lo) * b_lo
   k12..14: 1 * sqB_{hi,lo,lo2}
bf16*bf16 products are exact in fp32, so the dominant error is the dropped
sub-bf16 residue of the splits, ~1e-4 absolute on d^2.
"""

import numpy as np
import ml_dtypes

import concourse.bass as bass
import concourse.tile as tile
from concourse import bacc, mybir
from concourse.bass_utils import run_bass_kernel_spmd

BF16 = ml_dtypes.bfloat16
F32 = np.float32

K = 15            # augmented contraction rows
W = 192           # candidates per 32-query block ("unit")
BQ = 32           # queries per unit; four units share one matmul slot via
                  # PE column-tiling (tile_position=(0, 32*h))
UPB = 128 // BQ   # units per slot
PSW = 512         # PSUM bank stride in fp32 elements (one matmul <= 1 bank)
NSLOT1 = 64       # slots per core, round-1 NEFF (= 256 units)
NSLOT2 = 32       # slots per core, straggler NEFF (= 128 units)
GRP = 4           # slots fused per DMA + reduce (4 PSUM banks)
LEAF = 8          # k-d leaf size -> 4 sub-bboxes per 32-query unit
N_CORES = 8

# Sound per-query bound on device pairwise-d^2 arithmetic error:
# split residues ~2^-16*|a||b| + fp32 PSUM accumulation ~K*2^-23*|partials|.
PAD_SCALE = 2.5e-5
PAD_ABS = 2e-5


def _pad_q(sqA):
    return PAD_SCALE * sqA + PAD_ABS


# --------------------------------------------------------------------------
# Device program (static NEFFs, SPMD on 8 cores)
# --------------------------------------------------------------------------

def build_kernel(nslot):
    nc = bacc.Bacc("TRN2", target_bir_lowering=False, debug=False)

    lhsT_d = nc.dram_tensor("lhsT", [K, nslot * 128], mybir.dt.bfloat16,
                            kind="ExternalInput")
    rhs_d = nc.dram_tensor("rhs", [nslot // GRP, K, GRP * UPB * W],
                           mybir.dt.bfloat16, kind="ExternalInput")
    out_d = nc.dram_tensor("mins", [128, nslot], mybir.dt.float32,
                           kind="ExternalOutput")

    with tile.TileContext(nc) as tc:
        with (
            tc.tile_pool(name="io", bufs=1) as io_pool,
            tc.tile_pool(name="rh", bufs=4) as rh_pool,
            tc.tile_pool(name="ps", bufs=2, space=bass.MemorySpace.PSUM) as ps_pool,
        ):
            lhsT_s = io_pool.tile([K, nslot * 128], mybir.dt.bfloat16)
            nc.sync.dma_start(lhsT_s[:], lhsT_d[:])
            mins_all = io_pool.tile([128, nslot], mybir.dt.float32)

            for g in range(nslot // GRP):
                rt = rh_pool.tile([K, GRP * UPB * W], mybir.dt.bfloat16)
                nc.sync.dma_start(rt[:], rhs_d[g])
                # GRP banks; slot s in bank s, cols 0..W of the bank; the
                # two 64-query units of a slot land on partition halves via
                # PE column-tiling with their own rhs windows.
                ps = ps_pool.tile([128, GRP * PSW], mybir.dt.float32)
                for s in range(GRP):
                    m = g * GRP + s
                    for h in range(UPB):
                        nc.tensor.matmul(
                            ps[h * BQ : (h + 1) * BQ, s * PSW : s * PSW + W],
                            lhsT_s[:, m * 128 + h * BQ : m * 128 + (h + 1) * BQ],
                            rt[:, (s * UPB + h) * W : (s * UPB + h + 1) * W],
                            tile_position=(0, h * BQ),
                        )
                nc.vector.tensor_reduce(
                    mins_all[:, g * GRP : (g + 1) * GRP],
                    ps[:].rearrange("p (s n) -> p s n", n=PSW)[:, :, 0:W],
                    axis=mybir.AxisListType.X,
                    op=mybir.AluOpType.min,
                )

            nc.sync.dma_start(out_d[:], mins_all[:])

    nc.compile()
    return nc


_NC_CACHE = {}


def _get_nc(nslot):
    if nslot not in _NC_CACHE:
        _NC_CACHE[nslot] = build_kernel(nslot)
    return _NC_CACHE[nslot]


class _PjrtRunner:
    """Compile-once PJRT executor for one NEFF across the 8 cores.

    Mirrors bass2jax.run_bass_via_pjrt's multi-core path but holds the
    jitted shard_map so repeated waves skip XLA re-compilation.
    """

    def __init__(self, nc):
        import jax
        from concourse import bass2jax

        bass2jax.install_neuronx_cc_hook()
        self._jax = jax
        partition_name = (nc.partition_id_tensor.name
                          if nc.partition_id_tensor else None)
        in_names = []
        out_names = []
        out_avals = []
        zero_outs = []
        for alloc in nc.m.functions[0].allocations:
            if not isinstance(alloc, mybir.MemoryLocationSet):
                continue
            name = alloc.memorylocations[0].name
            if alloc.kind == "ExternalInput":
                if name != partition_name:
                    in_names.append(name)
            elif alloc.kind == "ExternalOutput":
                out_names.append(name)
                shape = tuple(alloc.tensor_shape)
                dtype = mybir.dt.np(alloc.dtype)
                out_avals.append(jax.core.ShapedArray(shape, dtype))
                zero_outs.append(np.zeros(shape, dtype))
        self.in_names = in_names
        self.out_names = out_names
        self.out_avals = out_avals
        self.zero_outs = zero_outs
        n_params = len(in_names)
        n_outs = len(out_names)
        all_in_names = list(in_names) + list(out_names)
        if partition_name is not None:
            all_in_names.append(partition_name)
        all_in_names = tuple(all_in_names)

        def _body(*args):
            operands = list(args)
            if partition_name is not None:
                operands.append(bass2jax.partition_id_tensor())
            outs = bass2jax._bass_exec_p.bind(
                *operands,
                out_avals=tuple(out_avals),
                in_names=all_in_names,
                out_names=tuple(out_names),
                lowering_input_output_aliases=(),
                sim_require_finite=True,
                sim_require_nnan=True,
                nc=nc,
            )
            return tuple(outs)

        devices = jax.devices()[:N_CORES]
        mesh = bass2jax.Mesh(np.asarray(devices), ("core",))
        P = bass2jax.PartitionSpec
        self._fn = jax.jit(
            bass2jax.shard_map(
                _body,
                mesh=mesh,
                in_specs=(P("core"),) * (n_params + n_outs),
                out_specs=(P("core"),) * n_outs,
                check_rep=False,
            ),
            donate_argnums=tuple(range(n_params, n_params + n_outs)),
            keep_unused=True,
        )

    def __call__(self, in_maps):
        np_ = np
        concat_in = [
            np_.concatenate([np_.asarray(m[name]) for m in in_maps], axis=0)
            for name in self.in_names
        ]
        concat_zeros = [
            np_.zeros((N_CORES * z.shape[0], *z.shape[1:]), z.dtype)
            for z in self.zero_outs
        ]
        out_arrs = self._fn(*concat_in, *concat_zeros)
        return [
            {
                name: np_.asarray(out_arrs[i]).reshape(
                    N_CORES, *self.out_avals[i].shape)[c]
                for i, name in enumerate(self.out_names)
            }
            for c in range(N_CORES)
        ]


_RUNNER_CACHE = {}


def _get_runner(nslot):
    if nslot not in _RUNNER_CACHE:
        _RUNNER_CACHE[nslot] = _PjrtRunner(_get_nc(nslot))
    return _RUNNER_CACHE[nslot]


class _WaveResults:
    def __init__(self, results):
        self.results = results


def run_wave(in_maps, nslot=NSLOT1, trace=False, **kw):
    if trace or kw:
        nc = _get_nc(nslot)
        return run_bass_kernel_spmd(nc, in_maps, list(range(N_CORES)),
                                    trace=trace, **kw)
    return _WaveResults(_get_runner(nslot)(in_maps))


# --------------------------------------------------------------------------
# Host-side prep
# --------------------------------------------------------------------------

def _split2(x):
    h = x.astype(BF16)
    l = (x - h.astype(F32)).astype(BF16)
    return h, l


def kd_order(P, leaf=LEAF):
    """Permutation grouping points into contiguous compact leaves of `leaf`."""
    out = []

    def rec(ids):
        if len(ids) <= leaf:
            out.append(ids)
            return
        pts = P[ids]
        ax = int(np.argmax(pts.max(0) - pts.min(0)))
        k = len(ids) // 2
        part = np.argpartition(pts[:, ax], k)
        rec(ids[part[:k]])
        rec(ids[part[k:]])

    rec(np.arange(len(P)))
    return np.concatenate(out)


_LEAF_D2_JIT = {}


def _leaf_d2_impl(lo, hi, B):
    import jax.numpy as jnp

    c = jnp.clip(B.T[:, None, :], lo.T[:, :, None], hi.T[:, :, None])
    t = B.T[:, None, :] - c                   # [3, nleaf, ncand]
    return (t * t).sum(0) * np.float32(1.0 - 1e-5)


def leaf_d2(q32, B32, leaf=LEAF):
    """[nleaf, ncand] fp32 lower bounds on min squared query-candidate dist.

    q32 is padded (by repeating the last point) to a multiple of `leaf`;
    the result is scaled by (1-1e-5) so fp32 rounding can never make it
    exceed the true distance.
    """
    import jax

    n = len(q32)
    if n % leaf:
        pad = leaf - n % leaf
        q32 = np.concatenate([q32, np.repeat(q32[-1:], pad, 0)])
    L = q32.reshape(-1, leaf, 3)
    lo = L.min(1)
    hi = L.max(1)
    key = (len(lo), len(B32))
    if key not in _LEAF_D2_JIT:
        cpu = jax.devices("cpu")[0]
        _LEAF_D2_JIT[key] = jax.jit(_leaf_d2_impl, device=cpu)
    return np.asarray(_LEAF_D2_JIT[key](lo, hi, B32))


class Job:
    """Host state for one (queries, candidates) job."""

    def __init__(self, Aq, Bc):
        self.N = len(Aq)
        self.order = kd_order(Aq)
        A = Aq[self.order]
        self.A32 = A
        self.B32 = Bc
        self.Ad = A.astype(np.float64)

        ah, al = _split2(A)
        m2ah = (ah.astype(F32) * -2.0).astype(BF16)
        m2al = (al.astype(F32) * -2.0).astype(BF16)
        L = np.empty((K, self.N), BF16)
        L[0:3] = m2ah.T
        L[3:6] = m2ah.T
        L[6:9] = m2al.T
        L[9:12] = m2al.T
        L[12:15] = np.ones((3, self.N), BF16)
        self.Lrows = L

        bh, bl = _split2(Bc)
        sqB = (Bc.astype(np.float64) ** 2).sum(-1).astype(F32)
        s0 = sqB.astype(BF16)
        r = sqB - s0.astype(F32)
        s1 = r.astype(BF16)
        s2 = (r - s1.astype(F32)).astype(BF16)
        R = np.empty((K, len(Bc)), BF16)
        R[0:3] = bh.T
        R[3:6] = bl.T
        R[6:9] = bh.T
        R[9:12] = bl.T
        R[12] = s0
        R[13] = s1
        R[14] = s2
        self.Rrows = R

        self.sqA = (self.Ad ** 2).sum(-1)  # permuted order, float64
        self.mins = np.full(self.N, np.inf)  # device value: d2 - sqA

        # Round-1 gather: per 64-query unit, W nearest-by-leaf-bbox
        # candidates; per leaf, coverage radius = min bound among
        # non-gathered.
        nblk = self.N // BQ
        nsub = BQ // LEAF
        d2 = leaf_d2(self.A32, self.B32)        # [nblk*nsub, ncand]
        ncand = len(self.B32)
        d2r = d2.reshape(nblk, nsub, ncand)
        d2b = d2r.min(1)                        # [nblk, ncand]
        part = np.argpartition(d2b, W, axis=1)
        self.sel = part[:, :W].copy()
        mask = np.zeros((nblk, ncand), bool)
        np.put_along_axis(mask, self.sel, True, axis=1)
        masked = np.where(mask[:, None, :], np.float32(np.inf), d2r)
        self.rcov = masked.min(2).reshape(-1).astype(np.float64)

    def round1_units(self):
        return [
            (np.arange(m * BQ, (m + 1) * BQ), self.sel[m])
            for m in range(self.N // BQ)
        ]

    def absorb(self, qidx, vals):
        np.minimum.at(self.mins, qidx, vals.astype(np.float64))

    def stragglers(self):
        """Per-query coverage check after round 1."""
        ub2 = np.maximum(self.mins + self.sqA, 0.0) + _pad_q(self.sqA)
        return np.where(ub2 > np.repeat(self.rcov, LEAF))[0]

    def round2_units(self, strag):
        """Conclusive follow-up units for straggler queries."""
        units = []
        if len(strag) == 0:
            return units
        sord = strag[kd_order(self.A32[strag])]
        for m0 in range(0, len(sord), BQ):
            ids = sord[m0 : m0 + BQ]
            d2b = leaf_d2(self.A32[ids], self.B32).min(0)
            ub2max = (np.maximum(self.mins[ids] + self.sqA[ids], 0.0)
                      + _pad_q(self.sqA[ids])).max()
            need = np.where(d2b <= ub2max)[0]
            if len(need) == 0:
                continue
            for c0 in range(0, len(need), W):
                cand = need[c0 : c0 + W]
                if len(cand) < W:
                    cand = np.concatenate(
                        [cand, np.full(W - len(cand), cand[0], np.int64)])
                units.append((ids, cand))
        return units


def _assemble_core(units, nslot):
    """Build one core's in_map from up to `2*nslot` (job, qidx, cand) units.

    Unit u maps to slot u//UPB, partition quarter u%UPB.
    """
    lhsT = np.zeros((K, nslot * 128), BF16)
    rhs = np.zeros((nslot // GRP, K, GRP * UPB * W), BF16)
    meta = []
    for u, (job, qidx, cand) in enumerate(units):
        s, h = divmod(u, UPB)
        ncol = len(qidx)
        c0 = s * 128 + h * BQ
        lhsT[:, c0 : c0 + ncol] = job.Lrows[:, qidx]
        g, r = divmod(s, GRP)
        rhs[g, :, (r * UPB + h) * W : (r * UPB + h + 1) * W] = job.Rrows[:, cand]
        meta.append((job, qidx, s, h))
    return {"lhsT": lhsT, "rhs": rhs}, meta


def _run_waves(all_units, nslot, trace=False):
    """Pack units onto cores, run as many 8-core waves as needed."""
    per_core = UPB * nslot
    per_wave = N_CORES * per_core
    for w0 in range(0, len(all_units), per_wave):
        wave = all_units[w0 : w0 + per_wave]
        in_maps = []
        metas = []
        for c in range(N_CORES):
            cunits = wave[c * per_core : (c + 1) * per_core]
            im, meta = _assemble_core(cunits, nslot)
            in_maps.append(im)
            metas.append(meta)
        res = run_wave(in_maps, nslot=nslot, trace=trace)
        for c in range(N_CORES):
            mins = res.results[c]["mins"]  # [128, nslot]
            for job, qidx, s, h in metas[c]:
                job.absorb(qidx, mins[h * BQ : h * BQ + len(qidx), s])


def kernel(xyz1, xyz2):
    xyz1 = np.asarray(xyz1, F32)
    xyz2 = np.asarray(xyz2, F32)
    nb = xyz1.shape[0]

    jobs = []
    for b in range(nb):
        jobs.append(Job(xyz1[b], xyz2[b]))
        jobs.append(Job(xyz2[b], xyz1[b]))

    # Round 1: job j's 128 units on core j (unit list is job-major)
    units1 = [(j, q, c) for j in jobs for q, c in j.round1_units()]
    _run_waves(units1, NSLOT1)

    # Round 2: conclusive straggler units (typically one short wave)
    units2 = [(j, q, c) for j in jobs for q, c in j.round2_units(j.stragglers())]
    if units2:
        nslot = NSLOT2 if len(units2) <= N_CORES * UPB * NSLOT2 else NSLOT1
        _run_waves(units2, nslot)

    total = 0.0
    for j in jobs:
        d = np.maximum(j.mins + j.sqA, 0.0)
        total += d.mean() / nb
    return np.asarray(total, dtype=F32)
